# revision 28
# baseline (speedup 1.0000x reference)
"""BitLinear158 (LayerNorm -> int8 fake-quant -> ternary matmul -> LayerNorm)
on 8 Trainium2 NeuronCores, data-parallel over tokens.

Math notes (vs the fp32 reference):
  - Input LayerNorm's rstd cancels inside the activation quantizer:
        q = round(xn / (max|xn|/127)) = round((x-mu) * 127 / max|x-mu|)
    so the input-side sqrt/reciprocal of the variance is never needed.
  - q in [-127,127] and ternary weights {-1,0,1} are exact in bf16, and the
    PE accumulates in fp32, so the matmul integer arithmetic is exact.
  - The final LayerNorm is invariant to the per-token positive scale
    (x_scale), up to the eps term (~1e-5 relative), so x_quant*x_scale is
    never materialized; weight_scale is applied in fp32 after the matmul.
  - round-half-to-even is implemented with the fp32 magic-number trick:
    t = fma(v, c, 1.5*2^23); q = t - 1.5*2^23.

Schedule notes (v2):
  - Weights stream as 16 per-kt chunk DMAs on a dedicated gpsimd SWDGE
    queue so block-0's transpose (sync HWDGE ring) is not serialized
    behind an 8 MiB transfer; first matmul starts ~13us in, not ~60us.
  - Per-block front end is split into feature halves (center, round,
    subtract, transpose) so the first matmuls of a block gate only on
    the first half.
  - The magic-subtract and the weight_scale multiply run on the
    otherwise idle GpSimd engine; Vector keeps the reductions/bn_stats.
  - Output LayerNorm apply + store are strip-wise (4 x 512) to shorten
    the end-of-kernel drain.
"""

from contextlib import ExitStack

import numpy as np
import ml_dtypes

N_CORES = 8
B, S, DIN, DOUT = 4, 4096, 2048, 2048
M_TOTAL = B * S
M_PER_CORE = M_TOTAL // N_CORES
P = 128
NBLK = M_PER_CORE // P          # token blocks per core
KT = DIN // P                   # contraction subtiles
NT = DOUT // 512                # psum bank tiles
EPS = 1e-5
MAGIC = float(np.float32(1.5 * 2 ** 23))

_CACHE = {}


def _build_nc(m_per_core=M_PER_CORE):
    key = ("nc", m_per_core)
    if key in _CACHE:
        return _CACHE[key]
    NBLK = m_per_core // P

    import concourse.bacc as bacc
    import concourse.tile as tile
    from concourse import mybir

    f32 = mybir.dt.float32
    f16 = mybir.dt.float16
    bf16 = mybir.dt.bfloat16
    X = mybir.AxisListType.X
    Identity = mybir.ActivationFunctionType.Identity
    Sqrt = mybir.ActivationFunctionType.Sqrt
    Max = mybir.AluOpType.max

    nc = bacc.Bacc("TRN2", target_bir_lowering=False, num_devices=N_CORES,
                   name="bitlinear158")
    xs = nc.dram_tensor("xs", [m_per_core, DIN], bf16, kind="ExternalInput")
    wt = nc.dram_tensor("wt", [DIN, DOUT], f16, kind="ExternalInput")
    out = nc.dram_tensor("out", [m_per_core, DOUT], f32, kind="ExternalOutput")

    with tile.TileContext(nc) as tc, ExitStack() as ctx:
        singles = ctx.enter_context(tc.tile_pool(name="singles", bufs=1))
        xpp = ctx.enter_context(tc.tile_pool(name="xpp", bufs=1))
        xp = ctx.enter_context(tc.tile_pool(name="xp", bufs=3))
        xcp = ctx.enter_context(tc.tile_pool(name="xcp", bufs=2))
        qp = ctx.enter_context(tc.tile_pool(name="qp", bufs=3))
        qtp = ctx.enter_context(tc.tile_pool(name="qtp", bufs=5))
        gp = ctx.enter_context(tc.tile_pool(name="gp", bufs=2))
        stp = ctx.enter_context(tc.tile_pool(name="stp", bufs=8))
        psp = ctx.enter_context(tc.tile_pool(name="psp", bufs=2, space="PSUM"))

        # Queue layout:
        #   qPool (gpsimd, SWDGE — non-blocking triggers): x loads + the 16
        #     weight chunks, interleaved so early x blocks are not starved
        #     and matmul kt gates only on weight chunk kt.
        #   qAct (scalar HWDGE): output stores only.  Large DMA batches on a
        #     HWDGE ring stall the issuing engine once the descriptor ring
        #     fills, so weights must NOT go here.
        #   qSP (sync HWDGE): wsc load, then the q transposes.
        w_k = [singles.tile([P, DOUT], f16, name=f"w_k{kt}")
               for kt in range(KT)]
        xpre = [xpp.tile([P, DIN], bf16, name=f"xpre{b}") for b in range(5)]
        for b in range(5):
            nc.gpsimd.dma_start(out=xpre[b], in_=xs[b * P:(b + 1) * P, :])
        for kt in range(KT):
            nc.gpsimd.dma_start(out=w_k[kt], in_=wt[kt * P:(kt + 1) * P, :])
        eps_t = singles.tile([P, 1], f32)
        nc.vector.memset(eps_t, EPS)
        magic_t = singles.tile([P, 1], f32)
        nc.vector.memset(magic_t, MAGIC)
        negmagic_t = singles.tile([P, 1], f32)
        nc.vector.memset(negmagic_t, -MAGIC)
        dum_l = singles.tile([P, P], f16)
        nc.vector.memset(dum_l, 0.0)
        dum_r = singles.tile([P, 512], f16)
        nc.vector.memset(dum_r, 0.0)

        H = DIN // 2
        for blk in range(NBLK):
            rows = slice(blk * P, (blk + 1) * P)
            if blk < 5:
                x_t = xpre[blk]
            else:
                x_t = xp.tile([P, DIN], bf16)
                nc.gpsimd.dma_start(out=x_t, in_=xs[rows, :])

            # ---- input LayerNorm + 8-bit absmax quant (rstd-free form) ----
            ssum = stp.tile([P, 1], f32)
            nc.vector.reduce_sum(out=ssum, in_=x_t, axis=X)
            negmu = stp.tile([P, 1], f32)
            nc.vector.tensor_scalar_mul(negmu, ssum, -1.0 / DIN)

            xc_t = xcp.tile([P, DIN], f32)          # x - mu (halves)
            am2 = stp.tile([P, 2], f32)
            for h in range(2):
                cols = slice(h * H, (h + 1) * H)
                nc.scalar.activation(out=xc_t[:, cols], in_=x_t[:, cols],
                                     func=Identity, bias=negmu, scale=1.0)
                nc.vector.tensor_reduce(out=am2[:, h:h + 1], in_=xc_t[:, cols],
                                        axis=X, op=Max,
                                        apply_absolute_value=True)
            amax = stp.tile([P, 1], f32)            # max |x - mu|
            nc.vector.tensor_reduce(out=amax, in_=am2, axis=X, op=Max)
            c127 = stp.tile([P, 1], f32)            # 127 / amax
            nc.vector.reciprocal(out=c127, in_=amax)
            nc.vector.tensor_scalar_mul(c127, c127, 127.0)

            # t = xc*c + MAGIC  (rounds to integer, RNE);  q = t - MAGIC
            # halves: the transpose of half h starts as soon as it is ready
            q_t = qp.tile([P, DIN], f16)
            qT3 = qtp.tile([P, KT, P], f16)
            for h in range(2):
                cols = slice(h * H, (h + 1) * H)
                nc.scalar.activation(out=xc_t[:, cols], in_=xc_t[:, cols],
                                     func=Identity, bias=magic_t, scale=c127)
                nc.vector.tensor_scalar(q_t[:, cols], xc_t[:, cols], MAGIC,
                                        None, op0=mybir.AluOpType.subtract)
                nc.sync.dma_start_transpose(
                    out=qT3[:, h * (KT // 2):(h + 1) * (KT // 2), :],
                    in_=q_t[:, cols])
            qT_t = qT3.rearrange("p kt m -> p (kt m)")

            # ---- exact integer matmul: psum = q @ (ternary*wsc).T ----
            ps = psp.tile([P, DOUT], f32)
            # Dummy matmuls keep the PE busy through the DMA-bound ramp
            # (blk 0: from t~0 before the first real matmul; blk 1-4: over
            # the front-end/w-stream waits) so HAM never re-throttles the
            # clock.  Results are overwritten by the real kt=0 start=True.
            n_warm = {0: 72}.get(blk, 0)
            for wu in range(n_warm):
                nc.tensor.matmul(ps[:, 0:512], lhsT=dum_l, rhs=dum_r,
                                 start=True, stop=True,
                                 skip_group_check=True)
            for kt in range(KT):
                for nt in range(NT):
                    ncols = slice(nt * 512, (nt + 1) * 512)
                    nc.tensor.matmul(ps[:, ncols],
                                     lhsT=qT_t[:, kt * P:(kt + 1) * P],
                                     rhs=w_k[kt][:, ncols],
                                     start=(kt == 0), stop=(kt == KT - 1))

            # ---- drain PSUM fast (scalar copy) so the next-next block's
            # matmuls get the banks back; stats/normalize use the copy ----
            g_t = gp.tile([P, DOUT], f32)
            nc.scalar.activation(out=g_t, in_=ps, func=Identity,
                                 bias=0.0, scale=1.0)
            st2 = stp.tile([P, 4, 6], f32)
            for sg in range(4):
                ncols = slice(sg * 512, (sg + 1) * 512)
                nc.vector.bn_stats(out=st2[:, sg, :], in_=g_t[:, ncols])
            mv2 = stp.tile([P, 2], f32)
            nc.vector.bn_aggr(out=mv2, in_=st2)

            rstd2 = stp.tile([P, 1], f32)
            nc.scalar.activation(out=rstd2, in_=mv2[:, 1:2], func=Sqrt,
                                 bias=eps_t, scale=1.0)
            nc.vector.reciprocal(out=rstd2, in_=rstd2)
            nb2 = stp.tile([P, 1], f32)
            nc.vector.tensor_scalar_mul(nb2, mv2[:, 0:1], -1.0)
            nc.vector.tensor_mul(nb2, nb2, rstd2)

            nc.scalar.activation(out=g_t, in_=g_t, func=Identity,
                                 bias=nb2, scale=rstd2)
            nc.scalar.dma_start(out=out[rows, :], in_=g_t)

    nc.compile()
    _CACHE[key] = nc
    return nc


def _prep_in_maps(x, weight_ternary, weight_scale):
    xs = np.ascontiguousarray(
        np.asarray(x, dtype=np.float32).reshape(M_TOTAL, DIN).astype(
            ml_dtypes.bfloat16))
    wsc = np.asarray(weight_scale, dtype=np.float32).reshape(1, DOUT)
    wt = np.ascontiguousarray(
        (np.asarray(weight_ternary).astype(np.float32).T * wsc).astype(
            np.float16))
    return [
        {"xs": np.ascontiguousarray(xs[c * M_PER_CORE:(c + 1) * M_PER_CORE]),
         "wt": wt}
        for c in range(N_CORES)
    ]


def run(x, weight_ternary, weight_scale, trace=False):
    from concourse.bass_utils import run_bass_kernel_spmd
    nc = _build_nc()
    in_maps = _prep_in_maps(x, weight_ternary, weight_scale)
    res = run_bass_kernel_spmd(nc, in_maps, core_ids=list(range(N_CORES)),
                               trace=trace)
    full = np.concatenate([res.results[c]["out"] for c in range(N_CORES)],
                          axis=0)
    return full.reshape(B, S, DOUT).astype(np.float32), res


def kernel(x, weight_ternary, weight_scale):
    out, _ = run(x, weight_ternary, weight_scale, trace=False)
    return out


# revision 29
# speedup vs baseline: 1.0319x; 1.0319x over previous
"""BitLinear158 (LayerNorm -> int8 fake-quant -> ternary matmul -> LayerNorm)
on 8 Trainium2 NeuronCores, data-parallel over tokens.

Math notes (vs the fp32 reference):
  - Input LayerNorm's rstd cancels inside the activation quantizer:
        q = round(xn / (max|xn|/127)) = round((x-mu) * 127 / max|x-mu|)
    so the input-side sqrt/reciprocal of the variance is never needed.
  - q in [-127,127] and ternary weights {-1,0,1} are exact in bf16, and the
    PE accumulates in fp32, so the matmul integer arithmetic is exact.
  - The final LayerNorm is invariant to the per-token positive scale
    (x_scale), up to the eps term (~1e-5 relative), so x_quant*x_scale is
    never materialized; weight_scale is applied in fp32 after the matmul.
  - round-half-to-even is implemented with the fp32 magic-number trick:
    t = fma(v, c, 1.5*2^23); q = t - 1.5*2^23.

Schedule notes (v2):
  - Weights stream as 16 per-kt chunk DMAs on a dedicated gpsimd SWDGE
    queue so block-0's transpose (sync HWDGE ring) is not serialized
    behind an 8 MiB transfer; first matmul starts ~13us in, not ~60us.
  - Per-block front end is split into feature halves (center, round,
    subtract, transpose) so the first matmuls of a block gate only on
    the first half.
  - The magic-subtract and the weight_scale multiply run on the
    otherwise idle GpSimd engine; Vector keeps the reductions/bn_stats.
  - Output LayerNorm apply + store are strip-wise (4 x 512) to shorten
    the end-of-kernel drain.
"""

from contextlib import ExitStack

import numpy as np
import ml_dtypes

N_CORES = 8
B, S, DIN, DOUT = 4, 4096, 2048, 2048
M_TOTAL = B * S
M_PER_CORE = M_TOTAL // N_CORES
P = 128
NBLK = M_PER_CORE // P          # token blocks per core
KT = DIN // P                   # contraction subtiles
NT = DOUT // 512                # psum bank tiles
EPS = 1e-5
MAGIC = float(np.float32(1.5 * 2 ** 23))

_CACHE = {}


def _build_nc(m_per_core=M_PER_CORE):
    key = ("nc", m_per_core)
    if key in _CACHE:
        return _CACHE[key]
    NBLK = m_per_core // P

    import concourse.bacc as bacc
    import concourse.tile as tile
    from concourse import mybir

    f32 = mybir.dt.float32
    f16 = mybir.dt.float16
    bf16 = mybir.dt.bfloat16
    f8 = mybir.dt.float8e4
    X = mybir.AxisListType.X
    Identity = mybir.ActivationFunctionType.Identity
    Sqrt = mybir.ActivationFunctionType.Sqrt
    Max = mybir.AluOpType.max

    nc = bacc.Bacc("TRN2", target_bir_lowering=False, num_devices=N_CORES,
                   name="bitlinear158")
    xs = nc.dram_tensor("xs", [m_per_core, DIN], bf16, kind="ExternalInput")
    wt = nc.dram_tensor("wt", [DIN, DOUT], f8, kind="ExternalInput")
    wsc = nc.dram_tensor("wsc", [P, DOUT], f32, kind="ExternalInput")
    out = nc.dram_tensor("out", [m_per_core, DOUT], f32, kind="ExternalOutput")

    with tile.TileContext(nc) as tc, ExitStack() as ctx:
        singles = ctx.enter_context(tc.tile_pool(name="singles", bufs=1))
        xpp = ctx.enter_context(tc.tile_pool(name="xpp", bufs=1))
        xp = ctx.enter_context(tc.tile_pool(name="xp", bufs=3))
        xcp = ctx.enter_context(tc.tile_pool(name="xcp", bufs=2))
        qp = ctx.enter_context(tc.tile_pool(name="qp", bufs=3))
        qtp = ctx.enter_context(tc.tile_pool(name="qtp", bufs=5))
        gp = ctx.enter_context(tc.tile_pool(name="gp", bufs=2))
        stp = ctx.enter_context(tc.tile_pool(name="stp", bufs=8))
        psp = ctx.enter_context(tc.tile_pool(name="psp", bufs=2, space="PSUM"))

        # Queue layout:
        #   qPool (gpsimd, SWDGE — non-blocking triggers): x loads + the 16
        #     weight chunks, interleaved so early x blocks are not starved
        #     and matmul kt gates only on weight chunk kt.
        #   qAct (scalar HWDGE): output stores only.  Large DMA batches on a
        #     HWDGE ring stall the issuing engine once the descriptor ring
        #     fills, so weights must NOT go here.
        #   qSP (sync HWDGE): wsc load, then the q transposes.
        w_k = [singles.tile([P, DOUT], f8, name=f"w_k{kt}")
               for kt in range(KT)]
        wsc_sb = singles.tile([P, DOUT], f32)
        nc.sync.dma_start(out=wsc_sb, in_=wsc.ap())
        xpre = [xpp.tile([P, DIN], bf16, name=f"xpre{b}") for b in range(5)]
        for b in range(5):
            nc.gpsimd.dma_start(out=xpre[b], in_=xs[b * P:(b + 1) * P, :])
        for kt in range(KT):
            nc.gpsimd.dma_start(out=w_k[kt], in_=wt[kt * P:(kt + 1) * P, :])
        eps_t = singles.tile([P, 1], f32)
        nc.vector.memset(eps_t, EPS)
        magic_t = singles.tile([P, 1], f32)
        nc.vector.memset(magic_t, MAGIC)
        negmagic_t = singles.tile([P, 1], f32)
        nc.vector.memset(negmagic_t, -MAGIC)
        dum_l = singles.tile([P, P], f16)
        nc.vector.memset(dum_l, 0.0)
        dum_r = singles.tile([P, 512], f16)
        nc.vector.memset(dum_r, 0.0)

        H = DIN // 2
        for blk in range(NBLK):
            rows = slice(blk * P, (blk + 1) * P)
            if blk < 5:
                x_t = xpre[blk]
            else:
                x_t = xp.tile([P, DIN], bf16)
                nc.gpsimd.dma_start(out=x_t, in_=xs[rows, :])

            # ---- input LayerNorm + 8-bit absmax quant (rstd-free form) ----
            ssum = stp.tile([P, 1], f32)
            nc.vector.reduce_sum(out=ssum, in_=x_t, axis=X)
            negmu = stp.tile([P, 1], f32)
            nc.vector.tensor_scalar_mul(negmu, ssum, -1.0 / DIN)

            xc_t = xcp.tile([P, DIN], f32)          # x - mu (halves)
            am2 = stp.tile([P, 2], f32)
            for h in range(2):
                cols = slice(h * H, (h + 1) * H)
                nc.scalar.activation(out=xc_t[:, cols], in_=x_t[:, cols],
                                     func=Identity, bias=negmu, scale=1.0)
                nc.vector.tensor_reduce(out=am2[:, h:h + 1], in_=xc_t[:, cols],
                                        axis=X, op=Max,
                                        apply_absolute_value=True)
            amax = stp.tile([P, 1], f32)            # max |x - mu|
            nc.vector.tensor_reduce(out=amax, in_=am2, axis=X, op=Max)
            c127 = stp.tile([P, 1], f32)            # 127 / amax
            nc.vector.reciprocal(out=c127, in_=amax)
            nc.vector.tensor_scalar_mul(c127, c127, 127.0)

            # t = xc*c + MAGIC  (rounds to integer, RNE);  q = t - MAGIC
            # halves: the transpose of half h starts as soon as it is ready
            q_t = qp.tile([P, DIN], f16)
            qT3 = qtp.tile([P, KT, P], f16)
            for h in range(2):
                cols = slice(h * H, (h + 1) * H)
                nc.scalar.activation(out=xc_t[:, cols], in_=xc_t[:, cols],
                                     func=Identity, bias=magic_t, scale=c127)
                nc.vector.tensor_scalar(q_t[:, cols], xc_t[:, cols], MAGIC,
                                        None, op0=mybir.AluOpType.subtract)
                nc.sync.dma_start_transpose(
                    out=qT3[:, h * (KT // 2):(h + 1) * (KT // 2), :],
                    in_=q_t[:, cols])
            qT_t = qT3.rearrange("p kt m -> p (kt m)")

            # ---- exact integer matmul: psum = q @ (ternary*wsc).T ----
            ps = psp.tile([P, DOUT], f32)
            # Dummy matmuls keep the PE busy through the DMA-bound ramp
            # (blk 0: from t~0 before the first real matmul; blk 1-4: over
            # the front-end/w-stream waits) so HAM never re-throttles the
            # clock.  Results are overwritten by the real kt=0 start=True.
            n_warm = {0: 72}.get(blk, 0)
            for wu in range(n_warm):
                nc.tensor.matmul(ps[:, 0:512], lhsT=dum_l, rhs=dum_r,
                                 start=True, stop=True,
                                 skip_group_check=True)
            for kt in range(KT):
                for nt in range(NT):
                    ncols = slice(nt * 512, (nt + 1) * 512)
                    nc.tensor.matmul(ps[:, ncols],
                                     lhsT=qT_t[:, kt * P:(kt + 1) * P],
                                     rhs=w_k[kt][:, ncols],
                                     start=(kt == 0), stop=(kt == KT - 1))

            # ---- g = psum * weight_scale (drains PSUM); LN stats ----
            g_t = gp.tile([P, DOUT], f32)
            nc.vector.tensor_mul(g_t, ps, wsc_sb)
            st2 = stp.tile([P, 4, 6], f32)
            for sg in range(4):
                ncols = slice(sg * 512, (sg + 1) * 512)
                nc.vector.bn_stats(out=st2[:, sg, :], in_=g_t[:, ncols])
            mv2 = stp.tile([P, 2], f32)
            nc.vector.bn_aggr(out=mv2, in_=st2)

            rstd2 = stp.tile([P, 1], f32)
            nc.scalar.activation(out=rstd2, in_=mv2[:, 1:2], func=Sqrt,
                                 bias=eps_t, scale=1.0)
            nc.vector.reciprocal(out=rstd2, in_=rstd2)
            nb2 = stp.tile([P, 1], f32)
            nc.vector.tensor_scalar_mul(nb2, mv2[:, 0:1], -1.0)
            nc.vector.tensor_mul(nb2, nb2, rstd2)

            nc.scalar.activation(out=g_t, in_=g_t, func=Identity,
                                 bias=nb2, scale=rstd2)
            nc.scalar.dma_start(out=out[rows, :], in_=g_t)

    nc.compile()
    _CACHE[key] = nc
    return nc


def _prep_in_maps(x, weight_ternary, weight_scale):
    xs = np.ascontiguousarray(
        np.asarray(x, dtype=np.float32).reshape(M_TOTAL, DIN).astype(
            ml_dtypes.bfloat16))
    wsc = np.ascontiguousarray(np.broadcast_to(
        np.asarray(weight_scale, dtype=np.float32).reshape(1, DOUT),
        (P, DOUT)))
    wt = np.ascontiguousarray(
        np.asarray(weight_ternary).astype(np.float32).T.astype(
            ml_dtypes.float8_e4m3fn))
    return [
        {"xs": np.ascontiguousarray(xs[c * M_PER_CORE:(c + 1) * M_PER_CORE]),
         "wt": wt, "wsc": wsc}
        for c in range(N_CORES)
    ]


def run(x, weight_ternary, weight_scale, trace=False):
    from concourse.bass_utils import run_bass_kernel_spmd
    nc = _build_nc()
    in_maps = _prep_in_maps(x, weight_ternary, weight_scale)
    res = run_bass_kernel_spmd(nc, in_maps, core_ids=list(range(N_CORES)),
                               trace=trace)
    full = np.concatenate([res.results[c]["out"] for c in range(N_CORES)],
                          axis=0)
    return full.reshape(B, S, DOUT).astype(np.float32), res


def kernel(x, weight_ternary, weight_scale):
    out, _ = run(x, weight_ternary, weight_scale, trace=False)
    return out
